# revision 1
# baseline (speedup 1.0000x reference)
"""Fused bf16 kernel: projections, attention, and output GEMM emitted as one
software-pipelined PE stream.

  startup:  weight/const DMAs; q/k proj chunk 0 (+RoPE); v tiles 0..3
  for j in 0..3:
    interleave( attention chunk j  (lookahead score/pv order, split
                DVE/GpSimd denominator chains, delayed den-matmul),
                fill = proj(j+1) (+RoPE), v blocks of j+1, wo(j-1) (+y DMA),
                       then xT(j+2) DMA issue )
  tail:     wo chunk 3

The attention stream alone stalls PE on score->exp->pv; the projection
matmuls are independent so the merged stream keeps PE dense.
PSUM tags: s(3, shared with den) pv(2) pjw(3, proj+v+wo) = 8 banks.
"""

import os
import sys

import numpy as np
import ml_dtypes

for _p in ("/opt/trn_rl_repo", "/root/.axon_site/_ro/trn_rl_repo"):
    if os.path.isdir(_p) and _p not in sys.path:
        sys.path.insert(0, _p)

import concourse.bass as bass
import concourse.tile as tile
from concourse import bacc
from concourse import mybir
from concourse import bass_utils

B, S, D, H = 2, 2048, 2048, 16
HD = 128
HPC = 4
CPB = 4
N_CORES = 8
NK = D // 128
NSQ = S // 512
NSK = S // 128
SCALE = float(1.0 / np.sqrt(np.float32(HD)))

F32 = mybir.dt.float32
BF16 = mybir.dt.bfloat16
NPBF = ml_dtypes.bfloat16

EXP = mybir.ActivationFunctionType.Exp
SWAP_MASK = [i ^ 1 for i in range(32)]

DEN_DELAY = 8


def _interleave(att_ops, fill_ops):
    na, nf = len(att_ops), len(fill_ops)
    fi = 0
    for ai, op in enumerate(att_ops):
        op()
        target = (ai + 1) * nf // max(na, 1)
        while fi < target:
            fill_ops[fi]()
            fi += 1
    while fi < nf:
        fill_ops[fi]()
        fi += 1


def _emit(tc):
    nc = tc.nc

    xT = nc.dram_tensor("xT", [D, S], BF16, kind="ExternalInput").ap()
    wqT = nc.dram_tensor("wqT", [D, HPC * HD], BF16, kind="ExternalInput").ap()
    wkT = nc.dram_tensor("wkT", [D, HPC * HD], BF16, kind="ExternalInput").ap()
    wvT = nc.dram_tensor("wvT", [D, HPC * HD], BF16, kind="ExternalInput").ap()
    woT = nc.dram_tensor("woT", [HPC * HD, D], BF16, kind="ExternalInput").ap()
    cosq = nc.dram_tensor("cosq", [HD, S], BF16, kind="ExternalInput").ap()
    sinq = nc.dram_tensor("sinq", [HD, S], BF16, kind="ExternalInput").ap()
    dmask = nc.dram_tensor("dmask", [4, 128, 512], BF16, kind="ExternalInput").ap()
    onesd = nc.dram_tensor("onesd", [128, 128], BF16, kind="ExternalInput").ap()
    y = nc.dram_tensor("y", [S, D], BF16, kind="ExternalOutput").ap()

    dma_a = nc.sync
    dma_b = nc.scalar
    TAGBUFS = {"pjw": 3, "s": 3, "pv": 2}

    # ---------------- pools ----------------
    consts = tc.alloc_tile_pool(name="consts", bufs=1)
    qk_pool = tc.alloc_tile_pool(name="qkp", bufs=HPC)
    qT = [qk_pool.tile([128, S], BF16, name=f"qT{h}", tag="qT") for h in range(HPC)]
    kT = [qk_pool.tile([128, S], BF16, name=f"kT{h}", tag="kT") for h in range(HPC)]

    ones_sq = consts.tile([128, 128], BF16, name="ones_sq")
    mask_sb = [
        consts.tile([128, 512], BF16, name=f"mask{m}", tag=f"mask{m}") for m in range(4)
    ]
    cos_sb = consts.tile([128, S], BF16, name="cos_sb")
    sin_sb = consts.tile([128, S], BF16, name="sin_sb")

    wq_pool = tc.alloc_tile_pool(name="wqp", bufs=NK)
    wk_pool = tc.alloc_tile_pool(name="wkp", bufs=NK)
    wv_pool = tc.alloc_tile_pool(name="wvp", bufs=NK)
    wo_pool = tc.alloc_tile_pool(name="wop", bufs=HPC)
    v_pool = tc.alloc_tile_pool(name="vp", bufs=NSK, side="right")
    v_sb = [
        v_pool.tile([128, HPC * HD], BF16, name=f"v{m}", tag="v") for m in range(NSK)
    ]
    oc_pool = tc.alloc_tile_pool(name="ocp", bufs=2 * HPC, side="right")
    xpool = tc.alloc_tile_pool(name="xp", bufs=32)
    tpool = tc.alloc_tile_pool(name="tqk", bufs=2)
    pp = tc.alloc_tile_pool(name="pp", bufs=6)
    small2 = tc.alloc_tile_pool(name="small2", bufs=2)
    ys_pool = tc.alloc_tile_pool(name="ysp", bufs=3)
    ps = tc.alloc_tile_pool(name="ps8", bufs=8, space="PSUM")

    # ---------------- startup DMAs ----------------
    wq_sb, wk_sb, wv_sb = [], [], []
    xs_boxes = {j: [] for j in range(NSQ)}
    for k in range(NK):
        wt = wq_pool.tile([128, HPC * HD], BF16, name=f"wq{k}", tag="wq")
        dma_a.dma_start(out=wt, in_=wqT[128 * k:128 * (k + 1), :])
        wq_sb.append(wt)
        xt = xpool.tile([128, 512], BF16, name=f"x_0_{k}", tag="xs")
        eng = dma_b if k % 2 == 0 else dma_a
        eng.dma_start(out=xt, in_=xT[128 * k:128 * (k + 1), 0:512])
        xs_boxes[0].append(xt)
        wt = wk_pool.tile([128, HPC * HD], BF16, name=f"wk{k}", tag="wk")
        dma_b.dma_start(out=wt, in_=wkT[128 * k:128 * (k + 1), :])
        wk_sb.append(wt)
    dma_a.dma_start(out=cos_sb, in_=cosq)
    dma_b.dma_start(out=sin_sb, in_=sinq)

    # ---------------- emission helpers ----------------
    def issue_xT(j):
        sl = slice(512 * j, 512 * (j + 1))
        for k in range(NK):
            xt = xpool.tile([128, 512], BF16, name=f"x_{j}_{k}", tag="xs")
            eng = dma_a if k % 2 == 0 else dma_b
            eng.dma_start(out=xt, in_=xT[128 * k:128 * (k + 1), sl])
            xs_boxes[j].append(xt)

    def proj_qk_ops(j, cycle=("pjw",)):
        """Closures for q/k projection chunk j (+RoPE fused on last mm)."""
        sl = slice(512 * j, 512 * (j + 1))
        ops = []
        ci = 0
        for w_sb, dsts, tagn in ((wq_sb, qT, "q"), (wk_sb, kT, "k")):
            for h in range(HPC):
                box = {}
                tag = cycle[ci % len(cycle)]
                ci += 1
                for k in range(NK):

                    def mm(k=k, w_sb=w_sb, h=h, j=j, tagn=tagn, box=box,
                           first=(k == 0), last=(k == NK - 1), tag=tag):
                        if first:
                            box["acc"] = ps.tile(
                                [128, 512], F32, name=f"a{tagn}{j}_{h}",
                                tag=tag, bufs=TAGBUFS[tag],
                            )
                        nc.tensor.matmul(
                            box["acc"], w_sb[k][:, 128 * h:128 * (h + 1)],
                            xs_boxes[j][k],
                            start=first, stop=last,
                        )

                    ops.append(mm)

                def rope(h=h, j=j, sl=sl, tagn=tagn, dsts=dsts, box=box):
                    acc = box["acc"]
                    shuf = tpool.tile(
                        [128, 512], F32, name=f"sh{tagn}_{j}_{h}", tag="shuf"
                    )
                    nc.vector.stream_shuffle(shuf, acc, SWAP_MASK)
                    t1 = tpool.tile([128, 512], BF16, name=f"t1{tagn}_{j}_{h}", tag="t1")
                    nc.gpsimd.tensor_mul(t1, shuf, sin_sb[:, sl])
                    t2 = tpool.tile([128, 512], BF16, name=f"t2{tagn}_{j}_{h}", tag="t2")
                    nc.vector.tensor_mul(t2, acc, cos_sb[:, sl])
                    nc.vector.tensor_add(dsts[h][:, sl], t1, t2)

                last_mm = ops.pop()

                def mm_rope(last_mm=last_mm, rope=rope):
                    last_mm()
                    rope()

                ops.append(mm_rope)
        return ops

    def v_ops(m, tag="pjw"):
        """Closures for v projection row-block m: natural [sk, hd] layout
        straight from the resident xT tiles (stationary = xT column slice,
        moving = wvT tile) -- no separate xV stream needed."""
        ops = []
        box = {}
        jj, loc = divmod(m, 4)
        cs = slice(128 * loc, 128 * (loc + 1))
        for k in range(NK):

            def mm(k=k, m=m, jj=jj, cs=cs, box=box, first=(k == 0),
                   last=(k == NK - 1), tag=tag):
                if first:
                    box["acc"] = ps.tile(
                        [128, HPC * HD], F32, name=f"av{m}", tag=tag,
                        bufs=TAGBUFS[tag],
                    )
                nc.tensor.matmul(
                    box["acc"], xs_boxes[jj][k][:, cs], wv_sb[k],
                    start=first, stop=last,
                )

            ops.append(mm)

        last_mm = ops.pop()

        def mm_copy(last_mm=last_mm, m=m, box=box):
            last_mm()
            nc.vector.tensor_copy(out=v_sb[m], in_=box["acc"])

        ops.append(mm_copy)
        return ops

    def wo_ops(j, out_c, cycle=("pjw",), alt_copy=False):
        """Closures for output GEMM of chunk j; y written in [128,512]
        blocks (per t,n) straight after each 4-matmul accumulation."""
        ops = []
        ci = 0
        for tloc in range(4):
            t = 4 * j + tloc
            for n in range(NSQ):
                box = {}
                tag = cycle[ci % len(cycle)]
                ci += 1
                for h in range(HPC):

                    def mm(h=h, n=n, tloc=tloc, out_c=out_c, box=box, t=t,
                           first=(h == 0), last=(h == HPC - 1), tag=tag):
                        if first:
                            box["acc"] = ps.tile(
                                [128, 512], F32, name=f"ay{t}_{n}", tag=tag,
                                bufs=TAGBUFS[tag],
                            )
                        nc.tensor.matmul(
                            box["acc"],
                            out_c[h][:, 128 * tloc:128 * (tloc + 1)],
                            wo_sb[h][:, 512 * n:512 * (n + 1)],
                            start=first, stop=last,
                        )
                        if last:
                            ys = ys_pool.tile(
                                [128, 512], BF16, name=f"ys{t}_{n}", tag="ys"
                            )
                            if alt_copy and (t + n) % 2 == 1:
                                nc.scalar.activation(
                                    ys, box["acc"],
                                    mybir.ActivationFunctionType.Copy,
                                )
                            else:
                                nc.vector.tensor_copy(out=ys, in_=box["acc"])
                            eng = dma_a if (t + n) % 2 == 0 else dma_b
                            eng.dma_start(
                                out=y[128 * t:128 * (t + 1),
                                      512 * n:512 * (n + 1)],
                                in_=ys,
                            )

                    ops.append(mm)
        return ops

    def att_ops(j, out_c):
        """Attention chunk j with lookahead order and delayed den ops."""
        sl = slice(512 * j, 512 * (j + 1))
        nsk = 4 * j + 4
        ops = []
        post = []
        for h in range(HPC):
            order = list(range(4 * j, nsk)) + list(range(0, 4 * j))
            offs = {0: 0, 1: 128, 2: 256, 3: 384}
            boxes = {}
            state = {}

            s_list, p_list = [], []
            for idx, i in enumerate(order):
                off = offs[i - 4 * j] if i >= 4 * j else 0
                cs = slice(off, 512)
                qs = slice(512 * j + off, 512 * (j + 1))

                def s_op(h=h, j=j, i=i, idx=idx, cs=cs, qs=qs, boxes=boxes,
                         diag=(i >= 4 * j), m=i - 4 * j):
                    s_ps = ps.tile(
                        [128, 512], F32, name=f"s{h}_{j}_{i}", tag="s", bufs=3
                    )
                    nc.tensor.matmul(
                        s_ps[:, cs], kT[h][:, 128 * i:128 * (i + 1)], qT[h][:, qs],
                        start=True, stop=True,
                    )
                    pt = pp.tile([128, 512], BF16, name=f"p{h}_{j}_{i}", tag="pt")
                    nc.scalar.activation(
                        pt[:, cs], s_ps[:, cs], EXP, bias=0.0, scale=SCALE
                    )
                    if diag:
                        nc.vector.tensor_mul(pt[:, cs], pt[:, cs], mask_sb[m][:, cs])
                    boxes[idx] = (pt, cs)

                def p_op(h=h, j=j, idx=idx, i=i, boxes=boxes, state=state,
                         nsk=nsk):
                    pt, cs = boxes[idx]
                    if idx == 0:
                        state["pv"] = ps.tile(
                            [128, 512], F32, name=f"pv{h}_{j}", tag="pv", bufs=2
                        )
                    nc.tensor.matmul(
                        state["pv"][:, cs], v_sb[i][:, 128 * h:128 * (h + 1)],
                        pt[:, cs],
                        start=(idx == 0), stop=(idx == nsk - 1),
                    )
                    # denominator chains (GpSimd evens / DVE odds)
                    if idx == nsk - 1:
                        nc.vector.tensor_add(
                            state["pacc_v"][:, cs], state["pacc_v"][:, cs],
                            pt[:, cs],
                        )
                        pacc_r = small2.tile(
                            [128, 512], BF16, name=f"par{h}_{j}", tag="paccr",
                            bufs=2,
                        )
                        nc.vector.tensor_add(
                            pacc_r, state["pacc_g"], state["pacc_v"]
                        )
                        state["pacc_r"] = pacc_r
                    elif idx == 0:
                        pg = small2.tile(
                            [128, 512], F32, name=f"pag{h}_{j}", tag="pacc_g",
                            bufs=2,
                        )
                        off = cs.start
                        if off > 0:
                            nc.gpsimd.memset(pg[:, 0:off], 0.0)
                        nc.gpsimd.tensor_copy(out=pg[:, cs], in_=pt[:, cs])
                        state["pacc_g"] = pg
                    elif idx == 1:
                        pv2 = small2.tile(
                            [128, 512], F32, name=f"pav{h}_{j}", tag="pacc_v",
                            bufs=2,
                        )
                        off = cs.start
                        if off > 0:
                            nc.vector.memset(pv2[:, 0:off], 0.0)
                        nc.vector.tensor_copy(out=pv2[:, cs], in_=pt[:, cs])
                        state["pacc_v"] = pv2
                    elif idx % 2 == 0:
                        nc.gpsimd.tensor_add(
                            state["pacc_g"][:, cs], state["pacc_g"][:, cs],
                            pt[:, cs],
                        )
                    else:
                        nc.vector.tensor_add(
                            state["pacc_v"][:, cs], state["pacc_v"][:, cs],
                            pt[:, cs],
                        )

                s_list.append(s_op)
                p_list.append(p_op)

            group = [s_list[0], s_list[1]]
            for idx in range(2, nsk):
                group.append(s_list[idx])
                group.append(p_list[idx - 2])
            group.append(p_list[nsk - 2])
            group.append(p_list[nsk - 1])

            def den_op(h=h, j=j, state=state, out_c=out_c):
                den = ps.tile([128, 512], F32, name=f"dn{h}_{j}", tag="s", bufs=3)
                nc.tensor.matmul(den, ones_sq, state["pacc_r"], start=True, stop=True)
                recip = small2.tile(
                    [128, 512], F32, name=f"rc{h}_{j}", tag="recip", bufs=2
                )
                scr = small2.tile(
                    [128, 512], F32, name=f"scx{h}_{j}", tag="scr", bufs=1
                )
                nc.vector.reciprocal_approx_accurate(recip, den, scr)
                nc.vector.tensor_mul(out_c[h], state["pv"], recip)

            post.append((len(ops) + len(group) + DEN_DELAY, den_op))
            ops.extend(group)
        for pos, op in sorted(post, key=lambda t: t[0], reverse=True):
            ops.insert(min(pos, len(ops)), op)
        return ops

    # ---------------- startup compute ----------------
    wo_sb = []
    for k in range(NK):
        wt = wv_pool.tile([128, HPC * HD], BF16, name=f"wv{k}", tag="wv")
        eng = dma_a if k % 2 == 0 else dma_b
        eng.dma_start(out=wt, in_=wvT[128 * k:128 * (k + 1), :])
        wv_sb.append(wt)
    issue_xT(1)
    dma_b.dma_start(out=ones_sq, in_=onesd)
    for m in range(4):
        dma_b.dma_start(out=mask_sb[m], in_=dmask[m])

    def issue_wo():
        for h in range(HPC):
            wt = wo_pool.tile([128, D], BF16, name=f"wo{h}", tag="wo")
            eng = dma_a if h % 2 == 0 else dma_b
            eng.dma_start(out=wt, in_=woT[128 * h:128 * (h + 1), :])
            wo_sb.append(wt)

    CYC = ("pjw", "s", "pv")
    for op in proj_qk_ops(0, cycle=CYC):
        op()
    for m in range(4):
        for op in v_ops(m, tag=CYC[m % len(CYC)]):
            op()

    out_cs = []
    for j in range(NSQ):
        out_c = [
            oc_pool.tile([128, 512], BF16, name=f"oc{j}_{h}", tag=f"oc{h}", bufs=2)
            for h in range(HPC)
        ]
        out_cs.append(out_c)
        fill = []
        if j < NSQ - 1:
            fill.extend(proj_qk_ops(j + 1))
            if j < NSQ - 2:

                def xt_issue(jn=j + 2):
                    issue_xT(jn)

                fill.insert(64, xt_issue)
            first_m = 4 * (j + 1)
            for m in range(first_m, first_m + 4):
                fill.extend(v_ops(m))
        if j == 0:
            fill.insert(80, lambda: issue_wo())
        if j > 0:
            fill.extend(wo_ops(j - 1, out_cs[j - 1]))
        _interleave(att_ops(j, out_c), fill)

    for op in wo_ops(
        NSQ - 1, out_cs[NSQ - 1], cycle=("pjw", "s", "pv"), alt_copy=True
    ):
        op()

    ps.release()
    ys_pool.release()
    small2.release()
    pp.release()
    tpool.release()
    xpool.release()
    oc_pool.release()
    v_pool.release()
    wo_pool.release()
    wv_pool.release()
    wk_pool.release()
    wq_pool.release()
    qk_pool.release()
    consts.release()


_PROGRAM = None


def build_program():
    global _PROGRAM
    if _PROGRAM is None:
        nc = bacc.Bacc("TRN2", target_bir_lowering=False, debug=False)
        with tile.TileContext(nc) as tc:
            _emit(tc)
        nc.compile()
        _PROGRAM = nc
    return _PROGRAM


def make_core_inputs(x, freqs_cos, freqs_sin, wq, wk, wv, wo):
    """Host-side sharding: returns list of 8 per-core input dicts."""
    x = np.asarray(x, dtype=np.float32)
    freqs_cos = np.asarray(freqs_cos, dtype=np.float32)
    freqs_sin = np.asarray(freqs_sin, dtype=np.float32)
    wq = np.asarray(wq, dtype=np.float32)
    wk = np.asarray(wk, dtype=np.float32)
    wv = np.asarray(wv, dtype=np.float32)
    wo = np.asarray(wo, dtype=np.float32)

    cosq = np.ascontiguousarray(np.repeat(freqs_cos.T, 2, axis=0)).astype(NPBF)
    sinq = np.ascontiguousarray(np.repeat(freqs_sin.T, 2, axis=0))
    sinq[0::2, :] *= -1.0  # even rows: -sin; odd rows: +sin
    sinq = sinq.astype(NPBF)

    skl = np.arange(128)[:, None]
    sql = np.arange(512)[None, :]
    dmask = np.stack([(128 * m + skl <= sql).astype(NPBF) for m in range(4)])

    onesd = np.ones((128, 128), dtype=NPBF)
    xTs = [np.ascontiguousarray(x[b].T).astype(NPBF) for b in range(B)]
    in_maps = []
    for c in range(N_CORES):
        b, g = divmod(c, CPB)
        hsl = slice(512 * g, 512 * (g + 1))
        in_maps.append(
            {
                "xT": xTs[b],
                "wqT": np.ascontiguousarray(wq[hsl, :].T).astype(NPBF),
                "wkT": np.ascontiguousarray(wk[hsl, :].T).astype(NPBF),
                "wvT": np.ascontiguousarray(wv[hsl, :].T).astype(NPBF),
                "woT": np.ascontiguousarray(wo[:, hsl].T).astype(NPBF),
                "cosq": cosq,
                "sinq": sinq,
                "dmask": dmask,
                "onesd": onesd,
            }
        )
    return in_maps


def run(inputs, trace=False, **spmd_kwargs):
    nc = build_program()
    in_maps = make_core_inputs(
        inputs["x"], inputs["freqs_cos"], inputs["freqs_sin"],
        inputs["wq"], inputs["wk"], inputs["wv"], inputs["wo"],
    )
    res = bass_utils.run_bass_kernel_spmd(
        nc, in_maps, list(range(N_CORES)), trace=trace, **spmd_kwargs
    )
    out = np.zeros((B, S, D), dtype=np.float32)
    for c in range(N_CORES):
        out[c // CPB] += np.asarray(res.results[c]["y"], dtype=np.float32)
    return out, res


def kernel(**inputs):
    out, _ = run(inputs, trace=False)
    return out


def simulate_core(core_idx, inputs):
    from concourse.bass_interp import CoreSim

    nc = build_program()
    in_maps = make_core_inputs(
        inputs["x"], inputs["freqs_cos"], inputs["freqs_sin"],
        inputs["wq"], inputs["wk"], inputs["wv"], inputs["wo"],
    )
    sim = CoreSim(nc)
    for name, arr in in_maps[core_idx].items():
        sim.tensor(name)[:] = arr
    sim.simulate()
    return np.array(sim.tensor("y"))



# revision 20
# speedup vs baseline: 1.6227x; 1.6227x over previous
"""Fused bf16 kernel: projections, attention, and output GEMM emitted as one
software-pipelined PE stream.

  startup:  weight/const DMAs; q/k proj chunk 0 (+RoPE); v tiles 0..3
  for j in 0..3:
    interleave( attention chunk j  (lookahead score/pv order, split
                DVE/GpSimd denominator chains, delayed den-matmul),
                fill = proj(j+1) (+RoPE), v blocks of j+1, wo(j-1) (+y DMA),
                       then xT(j+2) DMA issue )
  tail:     wo chunk 3

The attention stream alone stalls PE on score->exp->pv; the projection
matmuls are independent so the merged stream keeps PE dense.
PSUM tags: s(3, shared with den) pv(2) pjw(3, proj+v+wo) = 8 banks.
"""

import os
import sys

import numpy as np
import ml_dtypes

for _p in ("/opt/trn_rl_repo", "/root/.axon_site/_ro/trn_rl_repo"):
    if os.path.isdir(_p) and _p not in sys.path:
        sys.path.insert(0, _p)

import concourse.bass as bass
import concourse.tile as tile
from concourse import bacc
from concourse import mybir
from concourse import bass_utils

B, S, D, H = 2, 2048, 2048, 16
HD = 128
HPC = 4
CPB = 4
N_CORES = 8
NK = D // 128
NSQ = S // 512
NSK = S // 128
SCALE = float(1.0 / np.sqrt(np.float32(HD)))

F32 = mybir.dt.float32
BF16 = mybir.dt.bfloat16
NPBF = ml_dtypes.bfloat16

EXP = mybir.ActivationFunctionType.Exp
SWAP_MASK = [i ^ 1 for i in range(32)]

DEN_DELAY = int(os.environ.get("K_DEN_DELAY", "8"))
LOOKAHEAD = int(os.environ.get("K_LOOKAHEAD", "2"))


def _interleave(att_ops, fill_ops):
    na, nf = len(att_ops), len(fill_ops)
    fi = 0
    for ai, op in enumerate(att_ops):
        op()
        target = (ai + 1) * nf // max(na, 1)
        while fi < target:
            fill_ops[fi]()
            fi += 1
    while fi < nf:
        fill_ops[fi]()
        fi += 1


def _emit(tc, n_iter=1):
    nc = tc.nc

    xT = nc.dram_tensor("xT", [D, S], BF16, kind="ExternalInput").ap()
    wqT = nc.dram_tensor("wqT", [D, HPC * HD], BF16, kind="ExternalInput").ap()
    wkT = nc.dram_tensor("wkT", [D, HPC * HD], BF16, kind="ExternalInput").ap()
    wvT = nc.dram_tensor("wvT", [D, HPC * HD], BF16, kind="ExternalInput").ap()
    woT = nc.dram_tensor("woT", [HPC * HD, D], BF16, kind="ExternalInput").ap()
    cosq = nc.dram_tensor("cosq", [HD, S], BF16, kind="ExternalInput").ap()
    sinq = nc.dram_tensor("sinq", [HD, S], BF16, kind="ExternalInput").ap()
    dmask = nc.dram_tensor("dmask", [128, 128], BF16, kind="ExternalInput").ap()
    onesd = nc.dram_tensor("onesd", [128, 128], BF16, kind="ExternalInput").ap()
    y = nc.dram_tensor("y", [S, D], BF16, kind="ExternalOutput").ap()

    for _iter in range(n_iter):
        _emit_iter(tc, nc, xT, wqT, wkT, wvT, woT, cosq, sinq, dmask, onesd, y)


def _emit_iter(tc, nc, xT, wqT, wkT, wvT, woT, cosq, sinq, dmask, onesd, y):
    dma_a = nc.sync
    dma_b = nc.gpsimd
    TAGBUFS = {"pjw": 3, "s": 3, "pv": 2}

    # ---------------- pools ----------------
    consts = tc.alloc_tile_pool(name="consts", bufs=1)
    qk_pool = tc.alloc_tile_pool(name="qkp", bufs=HPC)
    qT = [qk_pool.tile([128, S], BF16, name=f"qT{h}", tag="qT") for h in range(HPC)]
    kT = [qk_pool.tile([128, S], BF16, name=f"kT{h}", tag="kT") for h in range(HPC)]

    ones_sq = consts.tile([128, 128], BF16, name="ones_sq")
    mask_sb = consts.tile([128, 128], BF16, name="mask_tri")
    cos_sb = consts.tile([128, S], BF16, name="cos_sb")
    sin_sb = consts.tile([128, S], BF16, name="sin_sb")

    wq_pool = tc.alloc_tile_pool(name="wqp", bufs=NK)
    wk_pool = tc.alloc_tile_pool(name="wkp", bufs=NK)
    wv_pool = tc.alloc_tile_pool(name="wvp", bufs=NK)
    wo_pool = tc.alloc_tile_pool(name="wop", bufs=HPC)
    v_pool = tc.alloc_tile_pool(name="vp", bufs=NSK, side="right")
    v_sb = [
        v_pool.tile([128, HPC * HD], BF16, name=f"v{m}", tag="v") for m in range(NSK)
    ]
    oc_pool = tc.alloc_tile_pool(name="ocp", bufs=2 * HPC, side="right")
    xpool = tc.alloc_tile_pool(name="xp", bufs=32)
    tpool = tc.alloc_tile_pool(name="tqk", bufs=2)
    pp = tc.alloc_tile_pool(name="pp", bufs=6)
    small2 = tc.alloc_tile_pool(name="small2", bufs=2)
    ys_pool = tc.alloc_tile_pool(name="ysp", bufs=3)
    ps = tc.alloc_tile_pool(name="ps8", bufs=8, space="PSUM")

    # ---------------- startup DMAs ----------------
    wq_sb, wk_sb, wv_sb = [], [], []
    xs_boxes = {j: [] for j in range(NSQ)}
    for k in range(NK):
        wt = wq_pool.tile([128, HPC * HD], BF16, name=f"wq{k}", tag="wq")
        dma_a.dma_start(out=wt, in_=wqT[128 * k:128 * (k + 1), :])
        wq_sb.append(wt)
        xt = xpool.tile([128, 512], BF16, name=f"x_0_{k}", tag="xs")
        eng = dma_b if k % 2 == 0 else dma_a
        eng.dma_start(out=xt, in_=xT[128 * k:128 * (k + 1), 0:512])
        xs_boxes[0].append(xt)
        wt = wk_pool.tile([128, HPC * HD], BF16, name=f"wk{k}", tag="wk")
        dma_b.dma_start(out=wt, in_=wkT[128 * k:128 * (k + 1), :])
        wk_sb.append(wt)
    dma_a.dma_start(out=cos_sb, in_=cosq)
    dma_b.dma_start(out=sin_sb, in_=sinq)

    # ---------------- emission helpers ----------------
    def issue_xT(j):
        sl = slice(512 * j, 512 * (j + 1))
        for k in range(NK):
            xt = xpool.tile([128, 512], BF16, name=f"x_{j}_{k}", tag="xs")
            eng = dma_a if k % 2 == 0 else dma_b
            eng.dma_start(out=xt, in_=xT[128 * k:128 * (k + 1), sl])
            xs_boxes[j].append(xt)

    def proj_qk_ops(j, cycle=("pjw",)):
        """Closures for q/k projection chunk j (+RoPE fused on last mm)."""
        sl = slice(512 * j, 512 * (j + 1))
        ops = []
        ci = 0
        for w_sb, dsts, tagn in ((wq_sb, qT, "q"), (wk_sb, kT, "k")):
            for h in range(HPC):
                box = {}
                tag = cycle[ci % len(cycle)]
                ci += 1
                for k in range(NK):

                    def mm(k=k, w_sb=w_sb, h=h, j=j, tagn=tagn, box=box,
                           first=(k == 0), last=(k == NK - 1), tag=tag):
                        if first:
                            box["acc"] = ps.tile(
                                [128, 512], F32, name=f"a{tagn}{j}_{h}",
                                tag=tag, bufs=TAGBUFS[tag],
                            )
                        nc.tensor.matmul(
                            box["acc"], w_sb[k][:, 128 * h:128 * (h + 1)],
                            xs_boxes[j][k],
                            start=first, stop=last,
                        )

                    ops.append(mm)

                def rope(h=h, j=j, sl=sl, tagn=tagn, dsts=dsts, box=box):
                    acc = box["acc"]
                    shuf = tpool.tile(
                        [128, 512], F32, name=f"sh{tagn}_{j}_{h}", tag="shuf"
                    )
                    nc.vector.stream_shuffle(shuf, acc, SWAP_MASK)
                    t1 = tpool.tile([128, 512], BF16, name=f"t1{tagn}_{j}_{h}", tag="t1")
                    nc.gpsimd.tensor_mul(t1, shuf, sin_sb[:, sl])
                    t2 = tpool.tile([128, 512], BF16, name=f"t2{tagn}_{j}_{h}", tag="t2")
                    nc.vector.tensor_mul(t2, acc, cos_sb[:, sl])
                    nc.vector.tensor_add(dsts[h][:, sl], t1, t2)

                last_mm = ops.pop()

                def mm_rope(last_mm=last_mm, rope=rope):
                    last_mm()
                    rope()

                ops.append(mm_rope)
        return ops

    def v_ops(m, tag="pjw"):
        """Closures for v projection row-block m: natural [sk, hd] layout
        straight from the resident xT tiles (stationary = xT column slice,
        moving = wvT tile) -- no separate xV stream needed."""
        ops = []
        box = {}
        jj, loc = divmod(m, 4)
        cs = slice(128 * loc, 128 * (loc + 1))
        for k in range(NK):

            def mm(k=k, m=m, jj=jj, cs=cs, box=box, first=(k == 0),
                   last=(k == NK - 1), tag=tag):
                if first:
                    box["acc"] = ps.tile(
                        [128, HPC * HD], F32, name=f"av{m}", tag=tag,
                        bufs=TAGBUFS[tag],
                    )
                nc.tensor.matmul(
                    box["acc"], xs_boxes[jj][k][:, cs], wv_sb[k],
                    start=first, stop=last,
                )

            ops.append(mm)

        last_mm = ops.pop()

        def mm_copy(last_mm=last_mm, m=m, box=box):
            last_mm()
            nc.vector.tensor_copy(out=v_sb[m], in_=box["acc"])

        ops.append(mm_copy)
        return ops

    def wo_ops(j, out_c, cycle=("pjw",), alt_copy=False):
        """Closures for output GEMM of chunk j; y written in [128,512]
        blocks (per t,n) straight after each 4-matmul accumulation."""
        ops = []
        ci = 0
        for tloc in range(4):
            t = 4 * j + tloc
            for n in range(NSQ):
                box = {}
                tag = cycle[ci % len(cycle)]
                ci += 1
                for h in range(HPC):

                    def mm(h=h, n=n, tloc=tloc, out_c=out_c, box=box, t=t,
                           first=(h == 0), last=(h == HPC - 1), tag=tag):
                        if first:
                            box["acc"] = ps.tile(
                                [128, 512], F32, name=f"ay{t}_{n}", tag=tag,
                                bufs=TAGBUFS[tag],
                            )
                        nc.tensor.matmul(
                            box["acc"],
                            out_c[h][:, 128 * tloc:128 * (tloc + 1)],
                            wo_sb[h][:, 512 * n:512 * (n + 1)],
                            start=first, stop=last,
                        )
                        if last:
                            ys = ys_pool.tile(
                                [128, 512], BF16, name=f"ys{t}_{n}", tag="ys"
                            )
                            if alt_copy:
                                # keep DVE free for the den/recip/out_c chain
                                nc.scalar.activation(
                                    ys, box["acc"],
                                    mybir.ActivationFunctionType.Copy,
                                )
                            else:
                                nc.vector.tensor_copy(out=ys, in_=box["acc"])
                            eng = dma_a if (t + n) % 2 == 0 else dma_b
                            eng.dma_start(
                                out=y[128 * t:128 * (t + 1),
                                      512 * n:512 * (n + 1)],
                                in_=ys,
                            )

                    ops.append(mm)
        return ops

    def att_ops(j, out_c):
        """Attention chunk j with lookahead order and delayed den ops."""
        sl = slice(512 * j, 512 * (j + 1))
        nsk = 4 * j + 4
        ops = []
        post = []
        for h in range(HPC):
            order = list(range(4 * j, nsk)) + list(range(0, 4 * j))
            offs = {0: 0, 1: 128, 2: 256, 3: 384}
            boxes = {}
            state = {}

            s_list, p_list = [], []
            for idx, i in enumerate(order):
                off = offs[i - 4 * j] if i >= 4 * j else 0
                cs = slice(off, 512)
                qs = slice(512 * j + off, 512 * (j + 1))

                def s_op(h=h, j=j, i=i, idx=idx, cs=cs, qs=qs, boxes=boxes,
                         diag=(i >= 4 * j), m=i - 4 * j):
                    s_ps = ps.tile(
                        [128, 512], F32, name=f"s{h}_{j}_{i}", tag="s", bufs=3
                    )
                    nc.tensor.matmul(
                        s_ps[:, cs], kT[h][:, 128 * i:128 * (i + 1)], qT[h][:, qs],
                        start=True, stop=True,
                    )
                    pt = pp.tile([128, 512], BF16, name=f"p{h}_{j}_{i}", tag="pt")
                    nc.scalar.activation(
                        pt[:, cs], s_ps[:, cs], EXP, bias=0.0, scale=SCALE
                    )
                    if diag:
                        # only the leading 128-col strip of a diagonal block
                        # is partially masked; later columns are fully visible
                        ms = slice(cs.start, cs.start + 128)
                        nc.vector.tensor_mul(pt[:, ms], pt[:, ms], mask_sb)
                    boxes[idx] = (pt, cs)

                def p_op(h=h, j=j, idx=idx, i=i, boxes=boxes, state=state,
                         nsk=nsk):
                    pt, cs = boxes[idx]
                    if idx == 0:
                        state["pv"] = ps.tile(
                            [128, 512], F32, name=f"pv{h}_{j}", tag="pv", bufs=2
                        )
                    nc.tensor.matmul(
                        state["pv"][:, cs], v_sb[i][:, 128 * h:128 * (h + 1)],
                        pt[:, cs],
                        start=(idx == 0), stop=(idx == nsk - 1),
                    )
                    # denominator chains (GpSimd evens / DVE odds)
                    if idx == nsk - 1:
                        nc.vector.tensor_add(
                            state["pacc_v"][:, cs], state["pacc_v"][:, cs],
                            pt[:, cs],
                        )
                        pacc_r = small2.tile(
                            [128, 512], BF16, name=f"par{h}_{j}", tag="paccr",
                            bufs=2,
                        )
                        nc.vector.tensor_add(
                            pacc_r, state["pacc_g"], state["pacc_v"]
                        )
                        state["pacc_r"] = pacc_r
                    elif idx == 0:
                        pg = small2.tile(
                            [128, 512], F32, name=f"pag{h}_{j}", tag="pacc_g",
                            bufs=2,
                        )
                        off = cs.start
                        if off > 0:
                            nc.vector.memset(pg[:, 0:off], 0.0)
                        nc.vector.tensor_copy(out=pg[:, cs], in_=pt[:, cs])
                        state["pacc_g"] = pg
                    elif idx == 1:
                        pv2 = small2.tile(
                            [128, 512], F32, name=f"pav{h}_{j}", tag="pacc_v",
                            bufs=2,
                        )
                        off = cs.start
                        if off > 0:
                            nc.vector.memset(pv2[:, 0:off], 0.0)
                        nc.vector.tensor_copy(out=pv2[:, cs], in_=pt[:, cs])
                        state["pacc_v"] = pv2
                    elif idx % 2 == 0:
                        nc.vector.tensor_add(
                            state["pacc_g"][:, cs], state["pacc_g"][:, cs],
                            pt[:, cs],
                        )
                    else:
                        nc.vector.tensor_add(
                            state["pacc_v"][:, cs], state["pacc_v"][:, cs],
                            pt[:, cs],
                        )

                s_list.append(s_op)
                p_list.append(p_op)

            la = min(LOOKAHEAD, nsk)
            group = [s_list[i] for i in range(la)]
            for idx in range(la, nsk):
                group.append(s_list[idx])
                group.append(p_list[idx - la])
            for idx in range(nsk - la, nsk):
                group.append(p_list[idx])

            def den_op(h=h, j=j, state=state, out_c=out_c):
                den = ps.tile([128, 512], F32, name=f"dn{h}_{j}", tag="s", bufs=3)
                nc.tensor.matmul(den, ones_sq, state["pacc_r"], start=True, stop=True)
                recip = small2.tile(
                    [128, 512], F32, name=f"rc{h}_{j}", tag="recip", bufs=2
                )
                scr = small2.tile(
                    [128, 512], F32, name=f"scx{h}_{j}", tag="scr", bufs=1
                )
                nc.vector.reciprocal_approx_accurate(recip, den, scr)
                nc.vector.tensor_mul(out_c[h], state["pv"], recip)

            post.append((len(ops) + len(group) + DEN_DELAY, den_op))
            ops.extend(group)
        for pos, op in sorted(post, key=lambda t: t[0], reverse=True):
            ops.insert(min(pos, len(ops)), op)
        return ops

    # ---------------- startup compute ----------------
    wo_sb = []
    for k in range(NK):
        wt = wv_pool.tile([128, HPC * HD], BF16, name=f"wv{k}", tag="wv")
        eng = dma_a if k % 2 == 0 else dma_b
        eng.dma_start(out=wt, in_=wvT[128 * k:128 * (k + 1), :])
        wv_sb.append(wt)
    issue_xT(1)
    dma_b.dma_start(out=ones_sq, in_=onesd)
    dma_b.dma_start(out=mask_sb, in_=dmask)

    def issue_wo():
        for h in range(HPC):
            wt = wo_pool.tile([128, D], BF16, name=f"wo{h}", tag="wo")
            eng = dma_a if h % 2 == 0 else dma_b
            eng.dma_start(out=wt, in_=woT[128 * h:128 * (h + 1), :])
            wo_sb.append(wt)

    CYC = ("pjw", "s", "pv")
    for op in proj_qk_ops(0, cycle=CYC):
        op()
    for m in range(4):
        for op in v_ops(m, tag=CYC[m % len(CYC)]):
            op()

    out_cs = []
    for j in range(NSQ):
        out_c = [
            oc_pool.tile([128, 512], BF16, name=f"oc{j}_{h}", tag=f"oc{h}", bufs=2)
            for h in range(HPC)
        ]
        out_cs.append(out_c)
        fill = []
        if j < NSQ - 1:
            fill.extend(proj_qk_ops(j + 1))
            if j < NSQ - 2:

                def xt_issue(jn=j + 2):
                    issue_xT(jn)

                fill.insert(64, xt_issue)
            first_m = 4 * (j + 1)
            for m in range(first_m, first_m + 4):
                fill.extend(v_ops(m))
        if j == 0:
            fill.insert(80, lambda: issue_wo())
        if j > 0:
            fill.extend(wo_ops(j - 1, out_cs[j - 1], alt_copy=True))
        _interleave(att_ops(j, out_c), fill)

    for op in wo_ops(
        NSQ - 1, out_cs[NSQ - 1], cycle=("pjw", "s", "pv"), alt_copy=True
    ):
        op()

    ps.release()
    ys_pool.release()
    small2.release()
    pp.release()
    tpool.release()
    xpool.release()
    oc_pool.release()
    v_pool.release()
    wo_pool.release()
    wv_pool.release()
    wk_pool.release()
    wq_pool.release()
    qk_pool.release()
    consts.release()


_PROGRAMS = {}


def build_program(n_iter=1):
    if n_iter not in _PROGRAMS:
        nc = bacc.Bacc("TRN2", target_bir_lowering=False, debug=False)
        with tile.TileContext(nc) as tc:
            _emit(tc, n_iter=n_iter)
        nc.compile()
        _PROGRAMS[n_iter] = nc
    return _PROGRAMS[n_iter]


def make_core_inputs(x, freqs_cos, freqs_sin, wq, wk, wv, wo):
    """Host-side sharding: returns list of 8 per-core input dicts."""
    x = np.asarray(x, dtype=np.float32)
    freqs_cos = np.asarray(freqs_cos, dtype=np.float32)
    freqs_sin = np.asarray(freqs_sin, dtype=np.float32)
    wq = np.asarray(wq, dtype=np.float32)
    wk = np.asarray(wk, dtype=np.float32)
    wv = np.asarray(wv, dtype=np.float32)
    wo = np.asarray(wo, dtype=np.float32)

    cosq = np.ascontiguousarray(np.repeat(freqs_cos.T, 2, axis=0)).astype(NPBF)
    sinq = np.ascontiguousarray(np.repeat(freqs_sin.T, 2, axis=0))
    sinq[0::2, :] *= -1.0  # even rows: -sin; odd rows: +sin
    sinq = sinq.astype(NPBF)

    skl = np.arange(128)[:, None]
    sql = np.arange(128)[None, :]
    dmask = (skl <= sql).astype(NPBF)

    onesd = np.ones((128, 128), dtype=NPBF)
    xTs = [np.ascontiguousarray(x[b].T).astype(NPBF) for b in range(B)]
    in_maps = []
    for c in range(N_CORES):
        b, g = divmod(c, CPB)
        hsl = slice(512 * g, 512 * (g + 1))
        in_maps.append(
            {
                "xT": xTs[b],
                "wqT": np.ascontiguousarray(wq[hsl, :].T).astype(NPBF),
                "wkT": np.ascontiguousarray(wk[hsl, :].T).astype(NPBF),
                "wvT": np.ascontiguousarray(wv[hsl, :].T).astype(NPBF),
                "woT": np.ascontiguousarray(wo[:, hsl].T).astype(NPBF),
                "cosq": cosq,
                "sinq": sinq,
                "dmask": dmask,
                "onesd": onesd,
            }
        )
    return in_maps


def run(inputs, trace=False, **spmd_kwargs):
    nc = build_program()
    in_maps = make_core_inputs(
        inputs["x"], inputs["freqs_cos"], inputs["freqs_sin"],
        inputs["wq"], inputs["wk"], inputs["wv"], inputs["wo"],
    )
    res = bass_utils.run_bass_kernel_spmd(
        nc, in_maps, list(range(N_CORES)), trace=trace, **spmd_kwargs
    )
    out = np.zeros((B, S, D), dtype=np.float32)
    for c in range(N_CORES):
        out[c // CPB] += np.asarray(res.results[c]["y"], dtype=np.float32)
    return out, res


def kernel(**inputs):
    out, _ = run(inputs, trace=False)
    return out


def simulate_core(core_idx, inputs):
    from concourse.bass_interp import CoreSim

    nc = build_program()
    in_maps = make_core_inputs(
        inputs["x"], inputs["freqs_cos"], inputs["freqs_sin"],
        inputs["wq"], inputs["wk"], inputs["wv"], inputs["wo"],
    )
    sim = CoreSim(nc)
    for name, arr in in_maps[core_idx].items():
        sim.tensor(name)[:] = arr
    sim.simulate()
    return np.array(sim.tensor("y"))

